# revision 19
# baseline (speedup 1.0000x reference)
"""Causal self-attention (B=4, T=2048, C=1024, NH=16) on 8 TRN2 NeuronCores.

Sharding (per spec hint): tensor-parallel over heads x data-parallel over batch.
Core i handles batch b = i//2 and head-group g = i%2 (8 heads each).
  - c_attn column-parallel: each core computes q,k,v for its 8 heads.
  - attention: fully local per core (its heads, its batch element).
  - c_proj row-parallel: each core computes a partial (T,C) output from its
    512 features; a 2-core ReduceScatter over pairs [[0,1],[2,3],[4,5],[6,7]]
    sums the partials, each core keeping half the rows. Host concatenates.

Device algorithm (per core), all matmuls bf16 with fp32 PSUM accumulation:
  xT (C,T) staged transposed by host.
  qT = wq^T @ xT, kT = wk^T @ xT   (feature-major, 4 chunks of 128)
  v  = x @ wv                      (token-major) + ones column per head
  per head pair (2fc, 2fc+1), per q-block Q (512 wide):
    s^T[kchunk] = kT_h^T @ qT_h    (K=64 contraction, row-tiled pair -> concurrent)
    p = exp(0.125 * s^T)  (ScalarE, bf16 out); causal-zeroed on GpSimd for
        diagonal chunks; fully-masked chunks skipped entirely.
    o^T[65,512] += v_aug_h^T @ p   (v_aug has a ones column -> row 64 = softmax
        denominators, fused into the same matmul)
    yT_h = o^T[0:64] * (1/o^T[64])  (PE K=1 broadcast of the reciprocal row)
  partial[T-block] = yT^T @ wp + 0.5*b_proj ; ReduceScatter(add) over the pair.
"""

import sys

if "/opt/trn_rl_repo" not in sys.path:
    sys.path.insert(0, "/opt/trn_rl_repo")

import numpy as np
import ml_dtypes

import concourse.bass as bass
import concourse.bacc as bacc
import concourse.mybir as mybir
import concourse.tile as tile
from concourse.bass import ts, ds
from concourse.bass_utils import run_bass_kernel_spmd

BF16 = ml_dtypes.bfloat16
N_CORES = 8
B, T, C = 4, 2048, 1024
NH, HS = 16, 64
H_LOC = NH // 2        # heads per core
F = H_LOC * HS         # 512 local qkv features
NFC = F // 128         # 4 feature chunks (one head pair each)
NKC = T // 128         # 16 key chunks
NQ = T // 512          # 4 query blocks
NCOL = C // 512        # 2 output column blocks
REPLICA_GROUPS = [[0, 1], [2, 3], [4, 5], [6, 7]]

FP32 = mybir.dt.float32
BF = mybir.dt.bfloat16


def _build_nc():
    # Bacc (not plain Bass): its compile() pipeline runs
    # generate_event_semaphores, which splits sync waits so no instruction
    # carries more than the hardware allows (walrus rejects >1 otherwise).
    nc = bacc.Bacc(None, target_bir_lowering=False, num_devices=N_CORES)

    xT = nc.dram_tensor("xT", [C, T], BF, kind="ExternalInput")
    wq = nc.dram_tensor("wq", [C, F], BF, kind="ExternalInput")
    wk = nc.dram_tensor("wk", [C, F], BF, kind="ExternalInput")
    wv = nc.dram_tensor("wv", [C, F], BF, kind="ExternalInput")
    bq = nc.dram_tensor("bq", [F], FP32, kind="ExternalInput")
    bk = nc.dram_tensor("bk", [F], FP32, kind="ExternalInput")
    bv = nc.dram_tensor("bv", [F], FP32, kind="ExternalInput")
    wp = nc.dram_tensor("wp", [F, C], BF, kind="ExternalInput")
    bp = nc.dram_tensor("bp", [C], FP32, kind="ExternalInput")
    out = nc.dram_tensor("out", [T // 2, C], FP32, kind="ExternalOutput")

    with tile.TileContext(nc) as tc:
        _body(tc, xT, wq, wk, wv, bq, bk, bv, wp, bp, out)
    nc.compile()
    return nc


def _body(tc, xT, wq, wk, wv, bq, bk, bv, wp, bp, out):
    nc = tc.nc
    import contextlib

    ctx = contextlib.ExitStack()
    with ctx:
        wpool = ctx.enter_context(tc.tile_pool(name="weights", bufs=1))
        apool = ctx.enter_context(tc.tile_pool(name="acts", bufs=1))
        ppool = ctx.enter_context(tc.tile_pool(name="ptiles", bufs=3))
        npool = ctx.enter_context(tc.tile_pool(name="norm", bufs=2))
        outp = ctx.enter_context(tc.tile_pool(name="outsb", bufs=3))
        # PSUM budget (8 banks): sAB [128,1024] x2 bufs = 4, oA/oB 1 bank x2 bufs = 4
        ps_s = ctx.enter_context(tc.tile_pool(name="ps_s", bufs=2, space="PSUM"))
        ps_o = ctx.enter_context(tc.tile_pool(name="ps_o", bufs=2, space="PSUM"))
        dpool = ctx.enter_context(tc.tile_pool(name="dram", bufs=1, space="DRAM"))

        # ---- stage inputs into SBUF ----
        x_sb = wpool.tile([128, C // 128, T], BF)
        nc.sync.dma_start(out=x_sb, in_=xT.rearrange("(ko p) t -> p ko t", p=128))
        wq_sb = wpool.tile([128, C // 128, F], BF)
        nc.sync.dma_start(out=wq_sb, in_=wq.rearrange("(ko p) f -> p ko f", p=128))
        wk_sb = wpool.tile([128, C // 128, F], BF)
        nc.sync.dma_start(out=wk_sb, in_=wk.rearrange("(ko p) f -> p ko f", p=128))
        wv_sb = wpool.tile([128, C // 128, F], BF)
        nc.sync.dma_start(out=wv_sb, in_=wv.rearrange("(ko p) f -> p ko f", p=128))
        wp_sb = wpool.tile([128, NFC, C], BF)
        nc.sync.dma_start(out=wp_sb, in_=wp.rearrange("(ko p) n -> p ko n", p=128))

        bq_sb = wpool.tile([128, NFC], FP32)
        nc.sync.dma_start(out=bq_sb, in_=bq.rearrange("(fo p) -> p fo", p=128))
        bk_sb = wpool.tile([128, NFC], FP32)
        nc.sync.dma_start(out=bk_sb, in_=bk.rearrange("(fo p) -> p fo", p=128))
        # broadcast biases across partitions (for token-major layouts)
        bv_bc = wpool.tile([128, F], FP32)
        nc.sync.dma_start(
            out=bv_bc,
            in_=bass.AP(tensor=bv.ap().tensor, offset=0, ap=[[0, 128], [1, F]]),
        )
        bp_bc = wpool.tile([128, C], FP32)
        nc.sync.dma_start(
            out=bp_bc,
            in_=bass.AP(tensor=bp.ap().tensor, offset=0, ap=[[0, 128], [1, C]]),
        )

        # ---- persistent activations ----
        qT_sb = apool.tile([128, NFC, T], BF)   # q, feature-major
        kT_sb = apool.tile([128, NFC, T], BF)   # k, feature-major
        # v token-major, 66-stride per head: cols 0:64 = v, col 64 = ones
        v_sb = apool.tile([128, NKC, H_LOC, 66], BF)
        nc.vector.memset(v_sb[:, :, :, 64:65], 1.0)
        yT_sb = apool.tile([128, NFC, T], BF)   # attention out, feature-major

        partial = dpool.tile([T, C], FP32)      # c_proj partial (pre-reduce)
        # per-Q-block ReduceScatter halves: core keeps [256,1024] per block
        rs_outs = [dpool.tile([256, C], FP32, name=f"rs_out{q}") for q in range(NQ)]

        KO = C // 128  # 8 contraction chunks for the projections

        # static causal mask tiles, one per diagonal offset j=0..3:
        # masks[j][kr, 512*h + qc] = 0 where qc >= 128*j + kr else -1e30
        # (both 512-halves identical -- they mask head A and head B scores
        # for the same key chunk)
        masks = []
        for j in range(4):
            m = wpool.tile([128, 1024], FP32, name=f"mask{j}")
            nc.gpsimd.memset(m, 0.0)
            for h in range(2):
                nc.gpsimd.affine_select(
                    out=m[:, ts(h, 512)],
                    in_=m[:, ts(h, 512)],
                    compare_op=mybir.AluOpType.is_ge,
                    fill=-1e30,
                    base=-128 * j,
                    channel_multiplier=-1,
                    pattern=[[1, 512]],
                )
            masks.append(m)

        # ---- phase 1: qT, kT (feature-major) ----
        for name, w_sb, b_sb, dst in (("q", wq_sb, bq_sb, qT_sb), ("k", wk_sb, bk_sb, kT_sb)):
            for fc in range(NFC):
                for tq2 in range(2):  # 1024-wide token spans
                    ps = ps_s.tile([128, 1024], FP32, tag="sAB")
                    for kc in range(KO):
                        for half in range(2):
                            nc.tensor.matmul(
                                ps[:, ts(half, 512)],
                                lhsT=w_sb[:, kc, ts(fc, 128)],
                                rhs=x_sb[:, kc, ds(tq2 * 1024 + half * 512, 512)],
                                start=(kc == 0),
                                stop=(kc == KO - 1),
                            )
                    nc.scalar.activation(
                        out=dst[:, fc, ts(tq2, 1024)],
                        in_=ps,
                        func=mybir.ActivationFunctionType.Identity,
                        bias=b_sb[:, fc : fc + 1],
                        scale=1.0,
                    )

        # ---- phase 1b: v (token-major) ----
        for tc_i in range(NKC):
            ps = ps_s.tile([128, 1024], FP32, tag="sAB")
            for kc in range(KO):
                nc.tensor.matmul(
                    ps[:, 0:512],
                    lhsT=x_sb[:, kc, ts(tc_i, 128)],
                    rhs=wv_sb[:, kc, :],
                    start=(kc == 0),
                    stop=(kc == KO - 1),
                )
            nc.vector.tensor_add(
                out=v_sb[:, tc_i, :, 0:64],
                in0=ps[:, 0:512].rearrange("p (h f) -> p h f", h=H_LOC),
                in1=bv_bc.rearrange("p (h f) -> p h f", h=H_LOC),
            )

        # ---- phase 2+3: attention per q-block; c_proj pipelined one block behind
        def attention_block(Q):
            nkc = 4 * Q + 4  # causal: only key chunks 0 .. 4Q+3 contribute
            for fc in range(NFC):  # head pair (2fc, 2fc+1)
                oA = ps_o.tile([65, 512], FP32, tag="oA")
                oB = ps_o.tile([65, 512], FP32, tag="oB")
                for kc in range(nkc):
                    # heads A and B share one 2-bank psum tile: A in cols
                    # 0:512 (array rows 0:64), B in 512:1024 (rows 64:128);
                    # the row-tiled pair runs concurrently on the PE.
                    sAB = ps_s.tile([128, 1024], FP32, tag="sAB")
                    nc.tensor.matmul(
                        sAB[:, 0:512],
                        lhsT=kT_sb[0:64, fc, ts(kc, 128)],
                        rhs=qT_sb[0:64, fc, ts(Q, 512)],
                        start=True,
                        stop=True,
                        tile_position=(0, 0),
                    )
                    nc.tensor.matmul(
                        sAB[:, ds(512, 512)],
                        lhsT=kT_sb[64:128, fc, ts(kc, 128)],
                        rhs=qT_sb[64:128, fc, ts(Q, 512)],
                        start=True,
                        stop=True,
                        tile_position=(64, 0),
                    )
                    if kc >= 4 * Q:  # crosses the causal boundary: pre-mask
                        nc.vector.tensor_add(
                            out=sAB, in0=sAB, in1=masks[kc - 4 * Q]
                        )
                    pAB = ppool.tile([128, 1024], BF, tag="pAB")
                    nc.scalar.activation(
                        out=pAB, in_=sAB, func=mybir.ActivationFunctionType.Exp,
                        scale=0.125,
                    )
                    nc.tensor.matmul(
                        oA,
                        lhsT=v_sb[:, kc, 2 * fc, 0:65],
                        rhs=pAB[:, 0:512],
                        start=(kc == 0),
                        stop=(kc == nkc - 1),
                    )
                    nc.tensor.matmul(
                        oB,
                        lhsT=v_sb[:, kc, 2 * fc + 1, 0:65],
                        rhs=pAB[:, ds(512, 512)],
                        start=(kc == 0),
                        stop=(kc == nkc - 1),
                    )
                # normalize: yT_h = oT[0:64] * (1 / oT[64]).
                # Everything off the TensorEngine queue: DVE approx
                # reciprocal + DMA partition-broadcast + DVE multiply.
                oA_sb = npool.tile([65, 512], FP32, tag="oAsb")
                oB_sb = npool.tile([65, 512], FP32, tag="oBsb")
                nc.vector.tensor_copy(out=oA_sb, in_=oA)
                nc.vector.tensor_copy(out=oB_sb, in_=oB)
                # custom-DVE reciprocal_approx_fast mishandles inputs at a
                # nonzero partition base -- stage row 64 down to partition 0
                rzA = npool.tile([1, 512], FP32, tag="rzA")
                rzB = npool.tile([1, 512], FP32, tag="rzB")
                nc.vector.tensor_copy(out=rzA, in_=oA_sb[64:65, :])
                nc.vector.tensor_copy(out=rzB, in_=oB_sb[64:65, :])
                rA = npool.tile([1, 512], FP32, tag="rA")
                rB = npool.tile([1, 512], FP32, tag="rB")
                nc.vector.reciprocal_approx_fast(out=rA, in_=rzA)
                nc.vector.reciprocal_approx_fast(out=rB, in_=rzB)
                # partition-broadcast via DRAM bounce (SBUF APs need nonzero
                # partition step; DRAM APs don't)
                rAd = dpool.tile([512], FP32, tag="rAd", bufs=2)
                rBd = dpool.tile([512], FP32, tag="rBd", bufs=2)
                nc.sync.dma_start(out=rAd[None, :], in_=rA)
                nc.sync.dma_start(out=rBd[None, :], in_=rB)
                bcA = npool.tile([64, 512], FP32, tag="bcA")
                bcB = npool.tile([64, 512], FP32, tag="bcB")
                nc.sync.dma_start(
                    out=bcA,
                    in_=bass.AP(tensor=rAd.tensor, offset=rAd.offset, ap=[[0, 64], [1, 512]]),
                )
                nc.sync.dma_start(
                    out=bcB,
                    in_=bass.AP(tensor=rBd.tensor, offset=rBd.offset, ap=[[0, 64], [1, 512]]),
                )
                # head A lives on partitions 0:64 of chunk fc
                nc.vector.tensor_mul(
                    out=yT_sb[0:64, fc, ts(Q, 512)], in0=oA_sb[0:64, :], in1=bcA
                )
                # head B must land on partitions 64:128 -> stage + DMA shift
                yB = npool.tile([64, 512], BF, tag="yB")
                nc.vector.tensor_mul(out=yB, in0=oB_sb[0:64, :], in1=bcB)
                nc.sync.dma_start(out=yT_sb[64:128, fc, ts(Q, 512)], in_=yB)

        def proj_block(Q):
            # c_proj for this block of 512 tokens, then pair-ReduceScatter
            for tb in range(4):
                trow = Q * 4 + tb
                ps = ps_s.tile([128, 1024], FP32, tag="sAB")
                for ncol in range(NCOL):
                    for fc in range(NFC):
                        nc.tensor.matmul(
                            ps[:, ts(ncol, 512)],
                            lhsT=yT_sb[:, fc, ts(trow, 128)],
                            rhs=wp_sb[:, fc, ts(ncol, 512)],
                            start=(fc == 0),
                            stop=(fc == NFC - 1),
                        )
                o_sb = outp.tile([128, 1024], FP32, tag="osb")
                nc.vector.tensor_add(out=o_sb, in0=ps, in1=bp_bc)
                nc.sync.dma_start(out=partial[ds(trow * 128, 128), :], in_=o_sb)

            # reduce this 512-token block across the batch pair while later
            # blocks still compute; each core keeps 256 of the 512 rows.
            nc.gpsimd.collective_compute(
                "ReduceScatter",
                mybir.AluOpType.add,
                replica_groups=REPLICA_GROUPS,
                ins=[partial[ds(Q * 512, 512), :]],
                outs=[rs_outs[Q][:]],
            )
            nc.sync.dma_start(
                out=out.ap()[ds(Q * 256, 256), :], in_=rs_outs[Q][:]
            )

        # software pipeline: proj(Q-1) issues behind attention(Q), so the PE
        # never waits on the normalization chain of the block it just finished
        for Q in range(NQ):
            attention_block(Q)
            if Q > 0:
                proj_block(Q - 1)
        proj_block(NQ - 1)


_NC_CACHE = None


def _get_nc():
    global _NC_CACHE
    if _NC_CACHE is None:
        _NC_CACHE = _build_nc()
    return _NC_CACHE


def kernel(x, w_attn, b_attn, w_proj, b_proj):
    x = np.asarray(x)
    w_attn = np.asarray(w_attn)
    b_attn = np.asarray(b_attn)
    w_proj = np.asarray(w_proj)
    b_proj = np.asarray(b_proj)

    nc = _get_nc()

    in_maps = []
    for i in range(N_CORES):
        b, g = i // 2, i % 2
        cols = slice(g * F, (g + 1) * F)
        in_maps.append(
            {
                "xT": np.ascontiguousarray(x[b].T).astype(BF16),
                "wq": np.ascontiguousarray(w_attn[:, g * F : (g + 1) * F]).astype(BF16),
                "wk": np.ascontiguousarray(
                    w_attn[:, C + g * F : C + (g + 1) * F]
                ).astype(BF16),
                "wv": np.ascontiguousarray(
                    w_attn[:, 2 * C + g * F : 2 * C + (g + 1) * F]
                ).astype(BF16),
                "bq": np.ascontiguousarray(b_attn[g * F : (g + 1) * F]).astype(
                    np.float32
                ),
                "bk": np.ascontiguousarray(b_attn[C + g * F : C + (g + 1) * F]).astype(
                    np.float32
                ),
                "bv": np.ascontiguousarray(
                    b_attn[2 * C + g * F : 2 * C + (g + 1) * F]
                ).astype(np.float32),
                "wp": np.ascontiguousarray(w_proj[g * F : (g + 1) * F, :]).astype(BF16),
                "bp": (b_proj * 0.5).astype(np.float32),
            }
        )

    global _last_in_maps
    _last_in_maps = in_maps  # stashed for external profiling harnesses
    res = run_bass_kernel_spmd(nc, in_maps, core_ids=list(range(N_CORES)))

    # Each core's "out" holds NQ blocks of 256 rows: block Q is the core's
    # ReduceScatter half of token rows [Q*512, (Q+1)*512) -- rank 0 (even
    # core) the first 256, rank 1 (odd core) the last 256.
    out = np.empty((B, T, C), dtype=np.float32)
    for b in range(B):
        even = res.results[2 * b]["out"].reshape(NQ, 256, C)
        odd = res.results[2 * b + 1]["out"].reshape(NQ, 256, C)
        blocks = out[b].reshape(NQ, 2, 256, C)
        blocks[:, 0] = even
        blocks[:, 1] = odd
    return out


# revision 22
# speedup vs baseline: 1.1084x; 1.1084x over previous
"""Causal self-attention (B=4, T=2048, C=1024, NH=16) on 8 TRN2 NeuronCores.

Sharding (per spec hint): tensor-parallel over heads x data-parallel over batch.
Core i handles batch b = i//2 and head-group g = i%2 (8 heads each).
  - c_attn column-parallel: each core computes q,k,v for its 8 heads.
  - attention: fully local per core (its heads, its batch element).
  - c_proj row-parallel: each core computes a partial (T,C) output from its
    512 features; a 2-core ReduceScatter over pairs [[0,1],[2,3],[4,5],[6,7]]
    sums the partials, each core keeping half the rows. Host concatenates.

Device algorithm (per core), all matmuls bf16 with fp32 PSUM accumulation:
  xT (C,T) staged transposed by host.
  qT = wq^T @ xT, kT = wk^T @ xT   (feature-major, 4 chunks of 128)
  v  = x @ wv                      (token-major) + ones column per head
  per head pair (2fc, 2fc+1), per q-block Q (512 wide):
    s^T[kchunk] = kT_h^T @ qT_h    (K=64 contraction, row-tiled pair -> concurrent)
    p = exp(0.125 * s^T)  (ScalarE, bf16 out); causal-zeroed on GpSimd for
        diagonal chunks; fully-masked chunks skipped entirely.
    o^T[65,512] += v_aug_h^T @ p   (v_aug has a ones column -> row 64 = softmax
        denominators, fused into the same matmul)
    yT_h = o^T[0:64] * (1/o^T[64])  (PE K=1 broadcast of the reciprocal row)
  partial[T-block] = yT^T @ wp + 0.5*b_proj ; ReduceScatter(add) over the pair.
"""

import sys

if "/opt/trn_rl_repo" not in sys.path:
    sys.path.insert(0, "/opt/trn_rl_repo")

import numpy as np
import ml_dtypes

import concourse.bass as bass
import concourse.bacc as bacc
import concourse.mybir as mybir
import concourse.tile as tile
from concourse.bass import ts, ds
from concourse.bass_utils import run_bass_kernel_spmd

BF16 = ml_dtypes.bfloat16
N_CORES = 8
B, T, C = 4, 2048, 1024
NH, HS = 16, 64
H_LOC = NH // 2        # heads per core
F = H_LOC * HS         # 512 local qkv features
NFC = F // 128         # 4 feature chunks (one head pair each)
NKC = T // 128         # 16 key chunks
NQ = T // 512          # 4 query blocks
NCOL = C // 512        # 2 output column blocks
REPLICA_GROUPS = [[0, 1], [2, 3], [4, 5], [6, 7]]

FP32 = mybir.dt.float32
BF = mybir.dt.bfloat16


def _build_nc():
    # Bacc (not plain Bass): its compile() pipeline runs
    # generate_event_semaphores, which splits sync waits so no instruction
    # carries more than the hardware allows (walrus rejects >1 otherwise).
    nc = bacc.Bacc(None, target_bir_lowering=False, num_devices=N_CORES)

    xT = nc.dram_tensor("xT", [C, T], BF, kind="ExternalInput")
    wq = nc.dram_tensor("wq", [C, F], BF, kind="ExternalInput")
    wk = nc.dram_tensor("wk", [C, F], BF, kind="ExternalInput")
    wv = nc.dram_tensor("wv", [C, F], BF, kind="ExternalInput")
    bq = nc.dram_tensor("bq", [F], FP32, kind="ExternalInput")
    bk = nc.dram_tensor("bk", [F], FP32, kind="ExternalInput")
    bv = nc.dram_tensor("bv", [F], FP32, kind="ExternalInput")
    wp = nc.dram_tensor("wp", [F, C], BF, kind="ExternalInput")
    bp = nc.dram_tensor("bp", [C], FP32, kind="ExternalInput")
    out = nc.dram_tensor("out", [T // 2, C], FP32, kind="ExternalOutput")

    with tile.TileContext(nc) as tc:
        _body(tc, xT, wq, wk, wv, bq, bk, bv, wp, bp, out)
    nc.compile()
    return nc


def _body(tc, xT, wq, wk, wv, bq, bk, bv, wp, bp, out):
    nc = tc.nc
    import contextlib

    ctx = contextlib.ExitStack()
    with ctx:
        wpool = ctx.enter_context(tc.tile_pool(name="weights", bufs=1))
        apool = ctx.enter_context(tc.tile_pool(name="acts", bufs=1))
        ppool = ctx.enter_context(tc.tile_pool(name="ptiles", bufs=3))
        npool = ctx.enter_context(tc.tile_pool(name="norm", bufs=2))
        outp = ctx.enter_context(tc.tile_pool(name="outsb", bufs=3))
        # PSUM budget (8 banks): sAB [128,1024] x2 bufs = 4, oA/oB 1 bank x2 bufs = 4
        ps_s = ctx.enter_context(tc.tile_pool(name="ps_s", bufs=2, space="PSUM"))
        ps_o = ctx.enter_context(tc.tile_pool(name="ps_o", bufs=2, space="PSUM"))
        dpool = ctx.enter_context(tc.tile_pool(name="dram", bufs=1, space="DRAM"))

        # ---- stage inputs into SBUF ----
        x_sb = wpool.tile([128, C // 128, T], BF)
        nc.sync.dma_start(out=x_sb, in_=xT.rearrange("(ko p) t -> p ko t", p=128))
        wq_sb = wpool.tile([128, C // 128, F], BF)
        nc.sync.dma_start(out=wq_sb, in_=wq.rearrange("(ko p) f -> p ko f", p=128))
        wk_sb = wpool.tile([128, C // 128, F], BF)
        nc.sync.dma_start(out=wk_sb, in_=wk.rearrange("(ko p) f -> p ko f", p=128))
        wv_sb = wpool.tile([128, C // 128, F], BF)
        nc.sync.dma_start(out=wv_sb, in_=wv.rearrange("(ko p) f -> p ko f", p=128))
        wp_sb = wpool.tile([128, NFC, C], BF)
        nc.sync.dma_start(out=wp_sb, in_=wp.rearrange("(ko p) n -> p ko n", p=128))

        bq_sb = wpool.tile([128, NFC], FP32)
        nc.sync.dma_start(out=bq_sb, in_=bq.rearrange("(fo p) -> p fo", p=128))
        bk_sb = wpool.tile([128, NFC], FP32)
        nc.sync.dma_start(out=bk_sb, in_=bk.rearrange("(fo p) -> p fo", p=128))
        # broadcast biases across partitions (for token-major layouts)
        bv_bc = wpool.tile([128, F], FP32)
        nc.sync.dma_start(
            out=bv_bc,
            in_=bass.AP(tensor=bv.ap().tensor, offset=0, ap=[[0, 128], [1, F]]),
        )
        bp_bc = wpool.tile([128, C], FP32)
        nc.sync.dma_start(
            out=bp_bc,
            in_=bass.AP(tensor=bp.ap().tensor, offset=0, ap=[[0, 128], [1, C]]),
        )

        # ---- persistent activations ----
        qT_sb = apool.tile([128, NFC, T], BF)   # q, feature-major
        kT_sb = apool.tile([128, NFC, T], BF)   # k, feature-major
        # v token-major, 66-stride per head: cols 0:64 = v, col 64 = ones
        v_sb = apool.tile([128, NKC, H_LOC, 66], BF)
        nc.vector.memset(v_sb[:, :, :, 64:65], 1.0)
        yT_sb = apool.tile([128, NFC, T], BF)   # attention out, feature-major

        partial = dpool.tile([T, C], FP32)      # c_proj partial (pre-reduce)
        # per-Q-block ReduceScatter halves: core keeps [256,1024] per block
        rs_outs = [dpool.tile([256, C], FP32, name=f"rs_out{q}") for q in range(NQ)]

        KO = C // 128  # 8 contraction chunks for the projections

        # ---- phase 1: qT, kT (feature-major) ----
        for name, w_sb, b_sb, dst in (("q", wq_sb, bq_sb, qT_sb), ("k", wk_sb, bk_sb, kT_sb)):
            for fc in range(NFC):
                for tq2 in range(2):  # 1024-wide token spans
                    ps = ps_s.tile([128, 1024], FP32, tag="sAB")
                    for kc in range(KO):
                        for half in range(2):
                            nc.tensor.matmul(
                                ps[:, ts(half, 512)],
                                lhsT=w_sb[:, kc, ts(fc, 128)],
                                rhs=x_sb[:, kc, ds(tq2 * 1024 + half * 512, 512)],
                                start=(kc == 0),
                                stop=(kc == KO - 1),
                            )
                    nc.scalar.activation(
                        out=dst[:, fc, ts(tq2, 1024)],
                        in_=ps,
                        func=mybir.ActivationFunctionType.Identity,
                        bias=b_sb[:, fc : fc + 1],
                        scale=1.0,
                    )

        # ---- phase 1b: v (token-major) ----
        for tc_i in range(NKC):
            ps = ps_s.tile([128, 1024], FP32, tag="sAB")
            for kc in range(KO):
                nc.tensor.matmul(
                    ps[:, 0:512],
                    lhsT=x_sb[:, kc, ts(tc_i, 128)],
                    rhs=wv_sb[:, kc, :],
                    start=(kc == 0),
                    stop=(kc == KO - 1),
                )
            nc.vector.tensor_add(
                out=v_sb[:, tc_i, :, 0:64],
                in0=ps[:, 0:512].rearrange("p (h f) -> p h f", h=H_LOC),
                in1=bv_bc.rearrange("p (h f) -> p h f", h=H_LOC),
            )

        # ---- phase 2+3: attention per q-block; c_proj pipelined one block behind
        def attention_block(Q):
            nkc = 4 * Q + 4  # causal: only key chunks 0 .. 4Q+3 contribute
            LAG = 2  # AV matmuls trail the QK/exp pipeline by this many chunks
            for fc in range(NFC):  # head pair (2fc, 2fc+1)
                oA = ps_o.tile([65, 512], FP32, tag="oA")
                oB = ps_o.tile([65, 512], FP32, tag="oB")
                pbuf = {}

                def emit_av(kc, oA=oA, oB=oB, nkc=nkc, fc=fc):
                    pAB = pbuf.pop(kc)
                    nc.tensor.matmul(
                        oA,
                        lhsT=v_sb[:, kc, 2 * fc, 0:65],
                        rhs=pAB[:, 0:512],
                        start=(kc == 0),
                        stop=(kc == nkc - 1),
                    )
                    nc.tensor.matmul(
                        oB,
                        lhsT=v_sb[:, kc, 2 * fc + 1, 0:65],
                        rhs=pAB[:, ds(512, 512)],
                        start=(kc == 0),
                        stop=(kc == nkc - 1),
                    )

                for kc in range(nkc):
                    # heads A and B share one 2-bank psum tile: A in cols
                    # 0:512 (array rows 0:64), B in 512:1024 (rows 64:128);
                    # the row-tiled pair runs concurrently on the PE.
                    sAB = ps_s.tile([128, 1024], FP32, tag="sAB")
                    nc.tensor.matmul(
                        sAB[:, 0:512],
                        lhsT=kT_sb[0:64, fc, ts(kc, 128)],
                        rhs=qT_sb[0:64, fc, ts(Q, 512)],
                        start=True,
                        stop=True,
                        tile_position=(0, 0),
                    )
                    nc.tensor.matmul(
                        sAB[:, ds(512, 512)],
                        lhsT=kT_sb[64:128, fc, ts(kc, 128)],
                        rhs=qT_sb[64:128, fc, ts(Q, 512)],
                        start=True,
                        stop=True,
                        tile_position=(64, 0),
                    )
                    pAB = ppool.tile([128, 1024], BF, tag="pAB", bufs=4)
                    nc.scalar.activation(
                        out=pAB, in_=sAB, func=mybir.ActivationFunctionType.Exp,
                        scale=0.125,
                    )
                    if kc >= 4 * Q:
                        # crosses the causal boundary: zero exp of masked
                        # scores (k_global > q_global) for both head halves
                        nc.gpsimd.affine_select(
                            out=pAB.rearrange("p (h q) -> p h q", h=2),
                            in_=pAB.rearrange("p (h q) -> p h q", h=2),
                            compare_op=mybir.AluOpType.is_ge,
                            fill=0.0,
                            base=512 * Q - 128 * kc,
                            channel_multiplier=-1,
                            pattern=[[0, 2], [1, 512]],
                        )
                    pbuf[kc] = pAB
                    if kc >= LAG:
                        emit_av(kc - LAG)
                for kc in range(max(0, nkc - LAG), nkc):
                    emit_av(kc)
                # normalize: yT_h = oT[0:64] * (1 / oT[64]).
                # Everything off the TensorEngine queue: DVE approx
                # reciprocal + DMA partition-broadcast + DVE multiply.
                oA_sb = npool.tile([65, 512], FP32, tag="oAsb")
                oB_sb = npool.tile([65, 512], FP32, tag="oBsb")
                nc.vector.tensor_copy(out=oA_sb, in_=oA)
                nc.vector.tensor_copy(out=oB_sb, in_=oB)
                # custom-DVE reciprocal_approx_fast mishandles inputs at a
                # nonzero partition base -- stage row 64 down to partition 0
                rzA = npool.tile([1, 512], FP32, tag="rzA")
                rzB = npool.tile([1, 512], FP32, tag="rzB")
                nc.vector.tensor_copy(out=rzA, in_=oA_sb[64:65, :])
                nc.vector.tensor_copy(out=rzB, in_=oB_sb[64:65, :])
                rA = npool.tile([1, 512], FP32, tag="rA")
                rB = npool.tile([1, 512], FP32, tag="rB")
                nc.vector.reciprocal_approx_fast(out=rA, in_=rzA)
                nc.vector.reciprocal_approx_fast(out=rB, in_=rzB)
                # partition-broadcast via DRAM bounce (SBUF APs need nonzero
                # partition step; DRAM APs don't)
                rAd = dpool.tile([512], FP32, tag="rAd", bufs=2)
                rBd = dpool.tile([512], FP32, tag="rBd", bufs=2)
                nc.sync.dma_start(out=rAd[None, :], in_=rA)
                nc.sync.dma_start(out=rBd[None, :], in_=rB)
                bcA = npool.tile([64, 512], FP32, tag="bcA")
                bcB = npool.tile([64, 512], FP32, tag="bcB")
                nc.sync.dma_start(
                    out=bcA,
                    in_=bass.AP(tensor=rAd.tensor, offset=rAd.offset, ap=[[0, 64], [1, 512]]),
                )
                nc.sync.dma_start(
                    out=bcB,
                    in_=bass.AP(tensor=rBd.tensor, offset=rBd.offset, ap=[[0, 64], [1, 512]]),
                )
                # head A lives on partitions 0:64 of chunk fc
                nc.vector.tensor_mul(
                    out=yT_sb[0:64, fc, ts(Q, 512)], in0=oA_sb[0:64, :], in1=bcA
                )
                # head B must land on partitions 64:128 -> stage + DMA shift
                yB = npool.tile([64, 512], BF, tag="yB")
                nc.vector.tensor_mul(out=yB, in0=oB_sb[0:64, :], in1=bcB)
                nc.sync.dma_start(out=yT_sb[64:128, fc, ts(Q, 512)], in_=yB)

        def proj_block(Q):
            # c_proj for this block of 512 tokens, then pair-ReduceScatter
            for tb in range(4):
                trow = Q * 4 + tb
                ps = ps_s.tile([128, 1024], FP32, tag="sAB")
                for ncol in range(NCOL):
                    for fc in range(NFC):
                        nc.tensor.matmul(
                            ps[:, ts(ncol, 512)],
                            lhsT=yT_sb[:, fc, ts(trow, 128)],
                            rhs=wp_sb[:, fc, ts(ncol, 512)],
                            start=(fc == 0),
                            stop=(fc == NFC - 1),
                        )
                o_sb = outp.tile([128, 1024], FP32, tag="osb")
                nc.vector.tensor_add(out=o_sb, in0=ps, in1=bp_bc)
                nc.sync.dma_start(out=partial[ds(trow * 128, 128), :], in_=o_sb)

            # reduce this 512-token block across the batch pair while later
            # blocks still compute; each core keeps 256 of the 512 rows.
            nc.gpsimd.collective_compute(
                "ReduceScatter",
                mybir.AluOpType.add,
                replica_groups=REPLICA_GROUPS,
                ins=[partial[ds(Q * 512, 512), :]],
                outs=[rs_outs[Q][:]],
            )
            # gpsimd (SWDGE) queue: this copy waits on the collective, and on
            # the sync HWDGE queue it would head-block latency-sensitive DMAs
            nc.gpsimd.dma_start(
                out=out.ap()[ds(Q * 256, 256), :], in_=rs_outs[Q][:]
            )

        # software pipeline: proj(Q-1) issues behind attention(Q), so the PE
        # never waits on the normalization chain of the block it just finished
        for Q in range(NQ):
            attention_block(Q)
            if Q > 0:
                proj_block(Q - 1)
        proj_block(NQ - 1)


_NC_CACHE = None


def _get_nc():
    global _NC_CACHE
    if _NC_CACHE is None:
        _NC_CACHE = _build_nc()
    return _NC_CACHE


def kernel(x, w_attn, b_attn, w_proj, b_proj):
    x = np.asarray(x)
    w_attn = np.asarray(w_attn)
    b_attn = np.asarray(b_attn)
    w_proj = np.asarray(w_proj)
    b_proj = np.asarray(b_proj)

    nc = _get_nc()

    in_maps = []
    for i in range(N_CORES):
        b, g = i // 2, i % 2
        cols = slice(g * F, (g + 1) * F)
        in_maps.append(
            {
                "xT": np.ascontiguousarray(x[b].T).astype(BF16),
                "wq": np.ascontiguousarray(w_attn[:, g * F : (g + 1) * F]).astype(BF16),
                "wk": np.ascontiguousarray(
                    w_attn[:, C + g * F : C + (g + 1) * F]
                ).astype(BF16),
                "wv": np.ascontiguousarray(
                    w_attn[:, 2 * C + g * F : 2 * C + (g + 1) * F]
                ).astype(BF16),
                "bq": np.ascontiguousarray(b_attn[g * F : (g + 1) * F]).astype(
                    np.float32
                ),
                "bk": np.ascontiguousarray(b_attn[C + g * F : C + (g + 1) * F]).astype(
                    np.float32
                ),
                "bv": np.ascontiguousarray(
                    b_attn[2 * C + g * F : 2 * C + (g + 1) * F]
                ).astype(np.float32),
                "wp": np.ascontiguousarray(w_proj[g * F : (g + 1) * F, :]).astype(BF16),
                "bp": (b_proj * 0.5).astype(np.float32),
            }
        )

    global _last_in_maps
    _last_in_maps = in_maps  # stashed for external profiling harnesses
    res = run_bass_kernel_spmd(nc, in_maps, core_ids=list(range(N_CORES)))

    # Each core's "out" holds NQ blocks of 256 rows: block Q is the core's
    # ReduceScatter half of token rows [Q*512, (Q+1)*512) -- rank 0 (even
    # core) the first 256, rank 1 (odd core) the last 256.
    out = np.empty((B, T, C), dtype=np.float32)
    for b in range(B):
        even = res.results[2 * b]["out"].reshape(NQ, 256, C)
        odd = res.results[2 * b + 1]["out"].reshape(NQ, 256, C)
        blocks = out[b].reshape(NQ, 2, 256, C)
        blocks[:, 0] = even
        blocks[:, 1] = odd
    return out


# revision 23
# speedup vs baseline: 1.1829x; 1.0672x over previous
"""Causal self-attention (B=4, T=2048, C=1024, NH=16) on 8 TRN2 NeuronCores.

Sharding (per spec hint): tensor-parallel over heads x data-parallel over batch.
Core i handles batch b = i//2 and head-group g = i%2 (8 heads each).
  - c_attn column-parallel: each core computes q,k,v for its 8 heads.
  - attention: fully local per core (its heads, its batch element).
  - c_proj row-parallel: each core computes a partial (T,C) output from its
    512 features; a 2-core ReduceScatter over pairs [[0,1],[2,3],[4,5],[6,7]]
    sums the partials, each core keeping half the rows. Host concatenates.

Device algorithm (per core), all matmuls bf16 with fp32 PSUM accumulation:
  xT (C,T) staged transposed by host.
  qT = wq^T @ xT, kT = wk^T @ xT   (feature-major, 4 chunks of 128)
  v  = x @ wv                      (token-major) + ones column per head
  per head pair (2fc, 2fc+1), per q-block Q (512 wide):
    s^T[kchunk] = kT_h^T @ qT_h    (K=64 contraction, row-tiled pair -> concurrent)
    p = exp(0.125 * s^T)  (ScalarE, bf16 out); causal-zeroed on GpSimd for
        diagonal chunks; fully-masked chunks skipped entirely.
    o^T[65,512] += v_aug_h^T @ p   (v_aug has a ones column -> row 64 = softmax
        denominators, fused into the same matmul)
    yT_h = o^T[0:64] * (1/o^T[64])  (PE K=1 broadcast of the reciprocal row)
  partial[T-block] = yT^T @ wp + 0.5*b_proj ; ReduceScatter(add) over the pair.
"""

import sys

if "/opt/trn_rl_repo" not in sys.path:
    sys.path.insert(0, "/opt/trn_rl_repo")

import numpy as np
import ml_dtypes

import concourse.bass as bass
import concourse.bacc as bacc
import concourse.mybir as mybir
import concourse.tile as tile
from concourse.bass import ts, ds
from concourse.bass_utils import run_bass_kernel_spmd

BF16 = ml_dtypes.bfloat16
N_CORES = 8
B, T, C = 4, 2048, 1024
NH, HS = 16, 64
H_LOC = NH // 2        # heads per core
F = H_LOC * HS         # 512 local qkv features
NFC = F // 128         # 4 feature chunks (one head pair each)
NKC = T // 128         # 16 key chunks
NQ = T // 512          # 4 query blocks
NCOL = C // 512        # 2 output column blocks
REPLICA_GROUPS = [[0, 1], [2, 3], [4, 5], [6, 7]]

FP32 = mybir.dt.float32
BF = mybir.dt.bfloat16


def _build_nc():
    # Bacc (not plain Bass): its compile() pipeline runs
    # generate_event_semaphores, which splits sync waits so no instruction
    # carries more than the hardware allows (walrus rejects >1 otherwise).
    nc = bacc.Bacc(None, target_bir_lowering=False, num_devices=N_CORES)

    xT = nc.dram_tensor("xT", [C, T], BF, kind="ExternalInput")
    wq = nc.dram_tensor("wq", [C, F], BF, kind="ExternalInput")
    wk = nc.dram_tensor("wk", [C, F], BF, kind="ExternalInput")
    wv = nc.dram_tensor("wv", [C, F], BF, kind="ExternalInput")
    bq = nc.dram_tensor("bq", [F], FP32, kind="ExternalInput")
    bk = nc.dram_tensor("bk", [F], FP32, kind="ExternalInput")
    bv = nc.dram_tensor("bv", [F], FP32, kind="ExternalInput")
    wp = nc.dram_tensor("wp", [F, C], BF, kind="ExternalInput")
    bp = nc.dram_tensor("bp", [C], FP32, kind="ExternalInput")
    out = nc.dram_tensor("out", [T // 2, C], FP32, kind="ExternalOutput")

    with tile.TileContext(nc) as tc:
        _body(tc, xT, wq, wk, wv, bq, bk, bv, wp, bp, out)
    nc.compile()
    return nc


def _body(tc, xT, wq, wk, wv, bq, bk, bv, wp, bp, out):
    nc = tc.nc
    import contextlib

    ctx = contextlib.ExitStack()
    with ctx:
        wpool = ctx.enter_context(tc.tile_pool(name="weights", bufs=1))
        apool = ctx.enter_context(tc.tile_pool(name="acts", bufs=1))
        ppool = ctx.enter_context(tc.tile_pool(name="ptiles", bufs=3))
        npool = ctx.enter_context(tc.tile_pool(name="norm", bufs=2))
        outp = ctx.enter_context(tc.tile_pool(name="outsb", bufs=3))
        # PSUM budget (8 banks): sAB [128,1024] x2 bufs = 4, oA/oB 1 bank x2 bufs = 4
        ps_s = ctx.enter_context(tc.tile_pool(name="ps_s", bufs=2, space="PSUM"))
        ps_o = ctx.enter_context(tc.tile_pool(name="ps_o", bufs=2, space="PSUM"))
        dpool = ctx.enter_context(tc.tile_pool(name="dram", bufs=1, space="DRAM"))

        # ---- stage inputs into SBUF ----
        x_sb = wpool.tile([128, C // 128, T], BF)
        nc.sync.dma_start(out=x_sb, in_=xT.rearrange("(ko p) t -> p ko t", p=128))
        wq_sb = wpool.tile([128, C // 128, F], BF)
        nc.sync.dma_start(out=wq_sb, in_=wq.rearrange("(ko p) f -> p ko f", p=128))
        wk_sb = wpool.tile([128, C // 128, F], BF)
        nc.sync.dma_start(out=wk_sb, in_=wk.rearrange("(ko p) f -> p ko f", p=128))
        wv_sb = wpool.tile([128, C // 128, F], BF)
        nc.sync.dma_start(out=wv_sb, in_=wv.rearrange("(ko p) f -> p ko f", p=128))
        wp_sb = wpool.tile([128, NFC, C], BF)
        nc.sync.dma_start(out=wp_sb, in_=wp.rearrange("(ko p) n -> p ko n", p=128))

        bq_sb = wpool.tile([128, NFC], FP32)
        nc.sync.dma_start(out=bq_sb, in_=bq.rearrange("(fo p) -> p fo", p=128))
        bk_sb = wpool.tile([128, NFC], FP32)
        nc.sync.dma_start(out=bk_sb, in_=bk.rearrange("(fo p) -> p fo", p=128))
        # broadcast biases across partitions (for token-major layouts)
        bv_bc = wpool.tile([128, F], FP32)
        nc.sync.dma_start(
            out=bv_bc,
            in_=bass.AP(tensor=bv.ap().tensor, offset=0, ap=[[0, 128], [1, F]]),
        )
        bp_bc = wpool.tile([128, C], FP32)
        nc.sync.dma_start(
            out=bp_bc,
            in_=bass.AP(tensor=bp.ap().tensor, offset=0, ap=[[0, 128], [1, C]]),
        )

        # ---- persistent activations ----
        qT_sb = apool.tile([128, NFC, T], BF)   # q, feature-major
        kT_sb = apool.tile([128, NFC, T], BF)   # k, feature-major
        # v token-major, 66-stride per head: cols 0:64 = v, col 64 = ones
        v_sb = apool.tile([128, NKC, H_LOC, 66], BF)
        nc.vector.memset(v_sb[:, :, :, 64:65], 1.0)
        yT_sb = apool.tile([128, NFC, T], BF)   # attention out, feature-major

        partial = dpool.tile([T, C], FP32)      # c_proj partial (pre-reduce)
        # per-Q-block ReduceScatter halves: core keeps [256,1024] per block
        rs_outs = [dpool.tile([256, C], FP32, name=f"rs_out{q}") for q in range(NQ)]

        KO = C // 128  # 8 contraction chunks for the projections

        # ---- phase 1: qT, kT (feature-major) ----
        for name, w_sb, b_sb, dst in (("q", wq_sb, bq_sb, qT_sb), ("k", wk_sb, bk_sb, kT_sb)):
            for fc in range(NFC):
                for tq2 in range(2):  # 1024-wide token spans
                    ps = ps_s.tile([128, 1024], FP32, tag="sAB")
                    for kc in range(KO):
                        for half in range(2):
                            nc.tensor.matmul(
                                ps[:, ts(half, 512)],
                                lhsT=w_sb[:, kc, ts(fc, 128)],
                                rhs=x_sb[:, kc, ds(tq2 * 1024 + half * 512, 512)],
                                start=(kc == 0),
                                stop=(kc == KO - 1),
                            )
                    nc.scalar.activation(
                        out=dst[:, fc, ts(tq2, 1024)],
                        in_=ps,
                        func=mybir.ActivationFunctionType.Identity,
                        bias=b_sb[:, fc : fc + 1],
                        scale=1.0,
                    )

        # ---- phase 1b: v (token-major) ----
        for tc_i in range(NKC):
            ps = ps_s.tile([128, 1024], FP32, tag="sAB")
            for kc in range(KO):
                nc.tensor.matmul(
                    ps[:, 0:512],
                    lhsT=x_sb[:, kc, ts(tc_i, 128)],
                    rhs=wv_sb[:, kc, :],
                    start=(kc == 0),
                    stop=(kc == KO - 1),
                )
            nc.vector.tensor_add(
                out=v_sb[:, tc_i, :, 0:64],
                in0=ps[:, 0:512].rearrange("p (h f) -> p h f", h=H_LOC),
                in1=bv_bc.rearrange("p (h f) -> p h f", h=H_LOC),
            )

        # ---- phase 2+3: attention per q-block; c_proj pipelined one block behind
        def attention_block(Q):
            nkc = 4 * Q + 4  # causal: only key chunks 0 .. 4Q+3 contribute
            LAG = 2  # AV matmuls trail the QK/exp pipeline by this many chunks
            for fc in range(NFC):  # head pair (2fc, 2fc+1)
                oA = ps_o.tile([65, 512], FP32, tag="oA")
                oB = ps_o.tile([65, 512], FP32, tag="oB")
                pbuf = {}

                def emit_av(kc, oA=oA, oB=oB, nkc=nkc, fc=fc):
                    pAB = pbuf.pop(kc)
                    nc.tensor.matmul(
                        oA,
                        lhsT=v_sb[:, kc, 2 * fc, 0:65],
                        rhs=pAB[:, 0:512],
                        start=(kc == 0),
                        stop=(kc == nkc - 1),
                    )
                    nc.tensor.matmul(
                        oB,
                        lhsT=v_sb[:, kc, 2 * fc + 1, 0:65],
                        rhs=pAB[:, ds(512, 512)],
                        start=(kc == 0),
                        stop=(kc == nkc - 1),
                    )

                for kc in range(nkc):
                    # heads A and B share one 2-bank psum tile: A in cols
                    # 0:512 (array rows 0:64), B in 512:1024 (rows 64:128);
                    # the row-tiled pair runs concurrently on the PE.
                    sAB = ps_s.tile([128, 1024], FP32, tag="sAB")
                    nc.tensor.matmul(
                        sAB[:, 0:512],
                        lhsT=kT_sb[0:64, fc, ts(kc, 128)],
                        rhs=qT_sb[0:64, fc, ts(Q, 512)],
                        start=True,
                        stop=True,
                        tile_position=(0, 0),
                    )
                    nc.tensor.matmul(
                        sAB[:, ds(512, 512)],
                        lhsT=kT_sb[64:128, fc, ts(kc, 128)],
                        rhs=qT_sb[64:128, fc, ts(Q, 512)],
                        start=True,
                        stop=True,
                        tile_position=(64, 0),
                    )
                    pAB = ppool.tile([128, 1024], BF, tag="pAB", bufs=4)
                    nc.scalar.activation(
                        out=pAB, in_=sAB, func=mybir.ActivationFunctionType.Exp,
                        scale=0.125,
                    )
                    if kc >= 4 * Q:
                        # crosses the causal boundary: zero exp of masked
                        # scores (k_global > q_global) for both head halves
                        nc.gpsimd.affine_select(
                            out=pAB.rearrange("p (h q) -> p h q", h=2),
                            in_=pAB.rearrange("p (h q) -> p h q", h=2),
                            compare_op=mybir.AluOpType.is_ge,
                            fill=0.0,
                            base=512 * Q - 128 * kc,
                            channel_multiplier=-1,
                            pattern=[[0, 2], [1, 512]],
                        )
                    pbuf[kc] = pAB
                    if kc >= LAG:
                        emit_av(kc - LAG)
                for kc in range(max(0, nkc - LAG), nkc):
                    emit_av(kc)
                # normalize: yT_h = oT[0:64] * (1 / oT[64]).
                # Everything off the TensorEngine queue: DVE approx
                # reciprocal + DMA partition-broadcast + DVE multiply.
                oA_sb = npool.tile([65, 512], FP32, tag="oAsb")
                oB_sb = npool.tile([65, 512], FP32, tag="oBsb")
                nc.vector.tensor_copy(out=oA_sb, in_=oA)
                nc.vector.tensor_copy(out=oB_sb, in_=oB)
                # custom-DVE reciprocal_approx_fast mishandles inputs at a
                # nonzero partition base -- stage row 64 down to partition 0
                rzA = npool.tile([1, 512], FP32, tag="rzA")
                rzB = npool.tile([1, 512], FP32, tag="rzB")
                nc.vector.tensor_copy(out=rzA, in_=oA_sb[64:65, :])
                nc.vector.tensor_copy(out=rzB, in_=oB_sb[64:65, :])
                rA = npool.tile([1, 512], FP32, tag="rA")
                rB = npool.tile([1, 512], FP32, tag="rB")
                nc.vector.reciprocal_approx_fast(out=rA, in_=rzA)
                nc.vector.reciprocal_approx_fast(out=rB, in_=rzB)
                # partition-broadcast via DRAM bounce (SBUF APs need nonzero
                # partition step; DRAM APs don't)
                rAd = dpool.tile([512], FP32, tag="rAd", bufs=2)
                rBd = dpool.tile([512], FP32, tag="rBd", bufs=2)
                nc.sync.dma_start(out=rAd[None, :], in_=rA)
                nc.sync.dma_start(out=rBd[None, :], in_=rB)
                bcA = npool.tile([64, 512], FP32, tag="bcA")
                bcB = npool.tile([64, 512], FP32, tag="bcB")
                nc.sync.dma_start(
                    out=bcA,
                    in_=bass.AP(tensor=rAd.tensor, offset=rAd.offset, ap=[[0, 64], [1, 512]]),
                )
                nc.sync.dma_start(
                    out=bcB,
                    in_=bass.AP(tensor=rBd.tensor, offset=rBd.offset, ap=[[0, 64], [1, 512]]),
                )
                # head A lives on partitions 0:64 of chunk fc
                nc.vector.tensor_mul(
                    out=yT_sb[0:64, fc, ts(Q, 512)], in0=oA_sb[0:64, :], in1=bcA
                )
                # head B must land on partitions 64:128 -> stage + DMA shift
                yB = npool.tile([64, 512], BF, tag="yB")
                nc.vector.tensor_mul(out=yB, in0=oB_sb[0:64, :], in1=bcB)
                nc.sync.dma_start(out=yT_sb[64:128, fc, ts(Q, 512)], in_=yB)

        def proj_block(Q):
            # c_proj for this block of 512 tokens, then pair-ReduceScatter
            for tb in range(4):
                trow = Q * 4 + tb
                ps = ps_s.tile([128, 1024], FP32, tag="sAB")
                for ncol in range(NCOL):
                    for fc in range(NFC):
                        nc.tensor.matmul(
                            ps[:, ts(ncol, 512)],
                            lhsT=yT_sb[:, fc, ts(trow, 128)],
                            rhs=wp_sb[:, fc, ts(ncol, 512)],
                            start=(fc == 0),
                            stop=(fc == NFC - 1),
                        )
                o_sb = outp.tile([128, 1024], FP32, tag="osb")
                nc.vector.tensor_add(out=o_sb, in0=ps, in1=bp_bc)
                nc.sync.dma_start(out=partial[ds(trow * 128, 128), :], in_=o_sb)

            # reduce this 512-token block across the batch pair while later
            # blocks still compute; each core keeps 256 of the 512 rows.
            nc.gpsimd.collective_compute(
                "ReduceScatter",
                mybir.AluOpType.add,
                replica_groups=REPLICA_GROUPS,
                ins=[partial[ds(Q * 512, 512), :]],
                outs=[rs_outs[Q][:]],
            )

        def out_copy(Q):
            # emitted one pipeline stage after RS(Q) was triggered, so the
            # wait is long-satisfied and doesn't head-block any engine queue
            nc.sync.dma_start(out=out.ap()[ds(Q * 256, 256), :], in_=rs_outs[Q][:])

        # software pipeline: proj(Q-1) issues behind attention(Q), so the PE
        # never waits on the normalization chain of the block it just finished
        for Q in range(NQ):
            attention_block(Q)
            if Q > 0:
                proj_block(Q - 1)
            if Q > 1:
                out_copy(Q - 2)
        proj_block(NQ - 1)
        out_copy(NQ - 2)
        out_copy(NQ - 1)


_NC_CACHE = None


def _get_nc():
    global _NC_CACHE
    if _NC_CACHE is None:
        _NC_CACHE = _build_nc()
    return _NC_CACHE


def kernel(x, w_attn, b_attn, w_proj, b_proj):
    x = np.asarray(x)
    w_attn = np.asarray(w_attn)
    b_attn = np.asarray(b_attn)
    w_proj = np.asarray(w_proj)
    b_proj = np.asarray(b_proj)

    nc = _get_nc()

    in_maps = []
    for i in range(N_CORES):
        b, g = i // 2, i % 2
        cols = slice(g * F, (g + 1) * F)
        in_maps.append(
            {
                "xT": np.ascontiguousarray(x[b].T).astype(BF16),
                "wq": np.ascontiguousarray(w_attn[:, g * F : (g + 1) * F]).astype(BF16),
                "wk": np.ascontiguousarray(
                    w_attn[:, C + g * F : C + (g + 1) * F]
                ).astype(BF16),
                "wv": np.ascontiguousarray(
                    w_attn[:, 2 * C + g * F : 2 * C + (g + 1) * F]
                ).astype(BF16),
                "bq": np.ascontiguousarray(b_attn[g * F : (g + 1) * F]).astype(
                    np.float32
                ),
                "bk": np.ascontiguousarray(b_attn[C + g * F : C + (g + 1) * F]).astype(
                    np.float32
                ),
                "bv": np.ascontiguousarray(
                    b_attn[2 * C + g * F : 2 * C + (g + 1) * F]
                ).astype(np.float32),
                "wp": np.ascontiguousarray(w_proj[g * F : (g + 1) * F, :]).astype(BF16),
                "bp": (b_proj * 0.5).astype(np.float32),
            }
        )

    global _last_in_maps
    _last_in_maps = in_maps  # stashed for external profiling harnesses
    res = run_bass_kernel_spmd(nc, in_maps, core_ids=list(range(N_CORES)))

    # Each core's "out" holds NQ blocks of 256 rows: block Q is the core's
    # ReduceScatter half of token rows [Q*512, (Q+1)*512) -- rank 0 (even
    # core) the first 256, rank 1 (odd core) the last 256.
    out = np.empty((B, T, C), dtype=np.float32)
    for b in range(B):
        even = res.results[2 * b]["out"].reshape(NQ, 256, C)
        odd = res.results[2 * b + 1]["out"].reshape(NQ, 256, C)
        blocks = out[b].reshape(NQ, 2, 256, C)
        blocks[:, 0] = even
        blocks[:, 1] = odd
    return out


# revision 26
# speedup vs baseline: 1.2320x; 1.0415x over previous
"""Causal self-attention (B=4, T=2048, C=1024, NH=16) on 8 TRN2 NeuronCores.

Sharding (per spec hint): tensor-parallel over heads x data-parallel over batch.
Core i handles batch b = i//2 and head-group g = i%2 (8 heads each).
  - c_attn column-parallel: each core computes q,k,v for its 8 heads.
  - attention: fully local per core (its heads, its batch element).
  - c_proj row-parallel: each core computes a partial (T,C) output from its
    512 features; a 2-core ReduceScatter over pairs [[0,1],[2,3],[4,5],[6,7]]
    sums the partials, each core keeping half the rows. Host concatenates.

Device algorithm (per core), all matmuls bf16 with fp32 PSUM accumulation:
  xT (C,T) staged transposed by host.
  qT = wq^T @ xT, kT = wk^T @ xT   (feature-major, 4 chunks of 128)
  v  = x @ wv                      (token-major) + ones column per head
  per head pair (2fc, 2fc+1), per q-block Q (512 wide):
    s^T[kchunk] = kT_h^T @ qT_h    (K=64 contraction, row-tiled pair -> concurrent)
    p = exp(0.125 * s^T)  (ScalarE, bf16 out); causal-zeroed on GpSimd for
        diagonal chunks; fully-masked chunks skipped entirely.
    o^T[65,512] += v_aug_h^T @ p   (v_aug has a ones column -> row 64 = softmax
        denominators, fused into the same matmul)
    yT_h = o^T[0:64] * (1/o^T[64])  (PE K=1 broadcast of the reciprocal row)
  partial[T-block] = yT^T @ wp + 0.5*b_proj ; ReduceScatter(add) over the pair.
"""

import sys

if "/opt/trn_rl_repo" not in sys.path:
    sys.path.insert(0, "/opt/trn_rl_repo")

import numpy as np
import ml_dtypes

import concourse.bass as bass
import concourse.bacc as bacc
import concourse.mybir as mybir
import concourse.tile as tile
from concourse.bass import ts, ds
from concourse.bass_utils import run_bass_kernel_spmd

BF16 = ml_dtypes.bfloat16
N_CORES = 8
B, T, C = 4, 2048, 1024
NH, HS = 16, 64
H_LOC = NH // 2        # heads per core
F = H_LOC * HS         # 512 local qkv features
NFC = F // 128         # 4 feature chunks (one head pair each)
NKC = T // 128         # 16 key chunks
NQ = T // 512          # 4 query blocks
NCOL = C // 512        # 2 output column blocks
REPLICA_GROUPS = [[0, 1], [2, 3], [4, 5], [6, 7]]

FP32 = mybir.dt.float32
BF = mybir.dt.bfloat16


def _build_nc():
    # Bacc (not plain Bass): its compile() pipeline runs
    # generate_event_semaphores, which splits sync waits so no instruction
    # carries more than the hardware allows (walrus rejects >1 otherwise).
    nc = bacc.Bacc(None, target_bir_lowering=False, num_devices=N_CORES)

    xT = nc.dram_tensor("xT", [C, T], BF, kind="ExternalInput")
    wq = nc.dram_tensor("wq", [C, F], BF, kind="ExternalInput")
    wk = nc.dram_tensor("wk", [C, F], BF, kind="ExternalInput")
    wv = nc.dram_tensor("wv", [C, F], BF, kind="ExternalInput")
    bq = nc.dram_tensor("bq", [F], FP32, kind="ExternalInput")
    bk = nc.dram_tensor("bk", [F], FP32, kind="ExternalInput")
    bv = nc.dram_tensor("bv", [F], FP32, kind="ExternalInput")
    wp = nc.dram_tensor("wp", [F, C], BF, kind="ExternalInput")
    bp = nc.dram_tensor("bp", [C], FP32, kind="ExternalInput")
    out = nc.dram_tensor("out", [T // 2, C], FP32, kind="ExternalOutput")

    with tile.TileContext(nc) as tc:
        _body(tc, xT, wq, wk, wv, bq, bk, bv, wp, bp, out)
    nc.compile()
    return nc


def _body(tc, xT, wq, wk, wv, bq, bk, bv, wp, bp, out):
    nc = tc.nc
    import contextlib

    ctx = contextlib.ExitStack()
    with ctx:
        wpool = ctx.enter_context(tc.tile_pool(name="weights", bufs=1))
        apool = ctx.enter_context(tc.tile_pool(name="acts", bufs=1))
        ppool = ctx.enter_context(tc.tile_pool(name="ptiles", bufs=3))
        npool = ctx.enter_context(tc.tile_pool(name="norm", bufs=2))
        outp = ctx.enter_context(tc.tile_pool(name="outsb", bufs=3))
        # PSUM budget (8 banks): sAB [128,1024] x3 bufs = 6, oA/oB 1 bank each = 2
        ps_s = ctx.enter_context(tc.tile_pool(name="ps_s", bufs=3, space="PSUM"))
        ps_o = ctx.enter_context(tc.tile_pool(name="ps_o", bufs=1, space="PSUM"))
        dpool = ctx.enter_context(tc.tile_pool(name="dram", bufs=1, space="DRAM"))

        # ---- stage inputs into SBUF ----
        x_sb = wpool.tile([128, C // 128, T], BF)
        nc.sync.dma_start(out=x_sb, in_=xT.rearrange("(ko p) t -> p ko t", p=128))
        wq_sb = wpool.tile([128, C // 128, F], BF)
        nc.sync.dma_start(out=wq_sb, in_=wq.rearrange("(ko p) f -> p ko f", p=128))
        wk_sb = wpool.tile([128, C // 128, F], BF)
        nc.sync.dma_start(out=wk_sb, in_=wk.rearrange("(ko p) f -> p ko f", p=128))
        wv_sb = wpool.tile([128, C // 128, F], BF)
        nc.sync.dma_start(out=wv_sb, in_=wv.rearrange("(ko p) f -> p ko f", p=128))
        wp_sb = wpool.tile([128, NFC, C], BF)
        nc.sync.dma_start(out=wp_sb, in_=wp.rearrange("(ko p) n -> p ko n", p=128))

        bq_sb = wpool.tile([128, NFC], FP32)
        nc.sync.dma_start(out=bq_sb, in_=bq.rearrange("(fo p) -> p fo", p=128))
        bk_sb = wpool.tile([128, NFC], FP32)
        nc.sync.dma_start(out=bk_sb, in_=bk.rearrange("(fo p) -> p fo", p=128))
        # broadcast biases across partitions (for token-major layouts)
        bv_bc = wpool.tile([128, F], FP32)
        nc.sync.dma_start(
            out=bv_bc,
            in_=bass.AP(tensor=bv.ap().tensor, offset=0, ap=[[0, 128], [1, F]]),
        )
        bp_bc = wpool.tile([128, C], FP32)
        nc.sync.dma_start(
            out=bp_bc,
            in_=bass.AP(tensor=bp.ap().tensor, offset=0, ap=[[0, 128], [1, C]]),
        )

        # ---- persistent activations ----
        qT_sb = apool.tile([128, NFC, T], BF)   # q, feature-major
        kT_sb = apool.tile([128, NFC, T], BF)   # k, feature-major
        # v token-major, 66-stride per head: cols 0:64 = v, col 64 = ones
        v_sb = apool.tile([128, NKC, H_LOC, 66], BF)
        nc.vector.memset(v_sb[:, :, :, 64:65], 1.0)
        yT_sb = apool.tile([128, NFC, T], BF)   # attention out, feature-major

        partial = dpool.tile([T, C], FP32)      # c_proj partial (pre-reduce)
        # per-Q-block ReduceScatter halves: core keeps [256,1024] per block
        rs_outs = [dpool.tile([256, C], FP32, name=f"rs_out{q}") for q in range(NQ)]

        KO = C // 128  # 8 contraction chunks for the projections

        # ---- phase 1: qT, kT (feature-major) ----
        for name, w_sb, b_sb, dst in (("q", wq_sb, bq_sb, qT_sb), ("k", wk_sb, bk_sb, kT_sb)):
            for fc in range(NFC):
                for tq2 in range(2):  # 1024-wide token spans
                    ps = ps_s.tile([128, 1024], FP32, tag="sAB")
                    for kc in range(KO):
                        for half in range(2):
                            nc.tensor.matmul(
                                ps[:, ts(half, 512)],
                                lhsT=w_sb[:, kc, ts(fc, 128)],
                                rhs=x_sb[:, kc, ds(tq2 * 1024 + half * 512, 512)],
                                start=(kc == 0),
                                stop=(kc == KO - 1),
                            )
                    nc.scalar.activation(
                        out=dst[:, fc, ts(tq2, 1024)],
                        in_=ps,
                        func=mybir.ActivationFunctionType.Identity,
                        bias=b_sb[:, fc : fc + 1],
                        scale=1.0,
                    )

        # ---- phase 1b: v (token-major) ----
        for tc_i in range(NKC):
            ps = ps_s.tile([128, 1024], FP32, tag="sAB")
            for kc in range(KO):
                nc.tensor.matmul(
                    ps[:, 0:512],
                    lhsT=x_sb[:, kc, ts(tc_i, 128)],
                    rhs=wv_sb[:, kc, :],
                    start=(kc == 0),
                    stop=(kc == KO - 1),
                )
            nc.vector.tensor_add(
                out=v_sb[:, tc_i, :, 0:64],
                in0=ps[:, 0:512].rearrange("p (h f) -> p h f", h=H_LOC),
                in1=bv_bc.rearrange("p (h f) -> p h f", h=H_LOC),
            )

        # ---- phase 2+3: attention per q-block; c_proj pipelined one block behind
        def attention_block(Q, interleave=None):
            nkc = 4 * Q + 4  # causal: only key chunks 0 .. 4Q+3 contribute
            LAG = 2  # AV matmuls trail the QK/exp pipeline by this many chunks
            for fc in range(NFC):  # head pair (2fc, 2fc+1)
                oA = ps_o.tile([65, 512], FP32, tag="oA")
                oB = ps_o.tile([65, 512], FP32, tag="oB")
                pbuf = {}

                def emit_av(kc, oA=oA, oB=oB, nkc=nkc, fc=fc):
                    pAB = pbuf.pop(kc)
                    nc.tensor.matmul(
                        oA,
                        lhsT=v_sb[:, kc, 2 * fc, 0:65],
                        rhs=pAB[:, 0:512],
                        start=(kc == 0),
                        stop=(kc == nkc - 1),
                    )
                    nc.tensor.matmul(
                        oB,
                        lhsT=v_sb[:, kc, 2 * fc + 1, 0:65],
                        rhs=pAB[:, ds(512, 512)],
                        start=(kc == 0),
                        stop=(kc == nkc - 1),
                    )

                for kc in range(nkc):
                    # heads A and B share one 2-bank psum tile: A in cols
                    # 0:512 (array rows 0:64), B in 512:1024 (rows 64:128);
                    # the row-tiled pair runs concurrently on the PE.
                    sAB = ps_s.tile([128, 1024], FP32, tag="sAB")
                    nc.tensor.matmul(
                        sAB[:, 0:512],
                        lhsT=kT_sb[0:64, fc, ts(kc, 128)],
                        rhs=qT_sb[0:64, fc, ts(Q, 512)],
                        start=True,
                        stop=True,
                        tile_position=(0, 0),
                    )
                    nc.tensor.matmul(
                        sAB[:, ds(512, 512)],
                        lhsT=kT_sb[64:128, fc, ts(kc, 128)],
                        rhs=qT_sb[64:128, fc, ts(Q, 512)],
                        start=True,
                        stop=True,
                        tile_position=(64, 0),
                    )
                    pAB = ppool.tile([128, 1024], BF, tag="pAB", bufs=4)
                    nc.scalar.activation(
                        out=pAB, in_=sAB, func=mybir.ActivationFunctionType.Exp,
                        scale=0.125,
                    )
                    if kc >= 4 * Q:
                        # crosses the causal boundary: zero exp of masked
                        # scores (k_global > q_global) for both head halves
                        nc.gpsimd.affine_select(
                            out=pAB.rearrange("p (h q) -> p h q", h=2),
                            in_=pAB.rearrange("p (h q) -> p h q", h=2),
                            compare_op=mybir.AluOpType.is_ge,
                            fill=0.0,
                            base=512 * Q - 128 * kc,
                            channel_multiplier=-1,
                            pattern=[[0, 2], [1, 512]],
                        )
                    pbuf[kc] = pAB
                    if kc >= LAG:
                        emit_av(kc - LAG)
                for kc in range(max(0, nkc - LAG), nkc):
                    emit_av(kc)
                # normalize: yT_h = oT[0:64] * (1 / oT[64]).
                # Everything off the TensorEngine queue: DVE approx
                # reciprocal + DMA partition-broadcast + DVE multiply.
                oA_sb = npool.tile([65, 512], FP32, tag="oAsb")
                oB_sb = npool.tile([65, 512], FP32, tag="oBsb")
                nc.vector.tensor_copy(out=oA_sb, in_=oA)
                nc.vector.tensor_copy(out=oB_sb, in_=oB)
                # custom-DVE reciprocal_approx_fast mishandles inputs at a
                # nonzero partition base -- stage row 64 down to partition 0
                rzA = npool.tile([1, 512], FP32, tag="rzA")
                rzB = npool.tile([1, 512], FP32, tag="rzB")
                nc.vector.tensor_copy(out=rzA, in_=oA_sb[64:65, :])
                nc.vector.tensor_copy(out=rzB, in_=oB_sb[64:65, :])
                rA = npool.tile([1, 512], FP32, tag="rA")
                rB = npool.tile([1, 512], FP32, tag="rB")
                nc.vector.reciprocal_approx_fast(out=rA, in_=rzA)
                nc.vector.reciprocal_approx_fast(out=rB, in_=rzB)
                # partition-broadcast via DRAM bounce (SBUF APs need nonzero
                # partition step; DRAM APs don't)
                rAd = dpool.tile([512], FP32, tag="rAd", bufs=2)
                rBd = dpool.tile([512], FP32, tag="rBd", bufs=2)
                nc.sync.dma_start(out=rAd[None, :], in_=rA)
                nc.sync.dma_start(out=rBd[None, :], in_=rB)
                bcA = npool.tile([64, 512], FP32, tag="bcA")
                bcB = npool.tile([64, 512], FP32, tag="bcB")
                nc.sync.dma_start(
                    out=bcA,
                    in_=bass.AP(tensor=rAd.tensor, offset=rAd.offset, ap=[[0, 64], [1, 512]]),
                )
                nc.sync.dma_start(
                    out=bcB,
                    in_=bass.AP(tensor=rBd.tensor, offset=rBd.offset, ap=[[0, 64], [1, 512]]),
                )
                # head A lives on partitions 0:64 of chunk fc
                nc.vector.tensor_mul(
                    out=yT_sb[0:64, fc, ts(Q, 512)], in0=oA_sb[0:64, :], in1=bcA
                )
                # head B must land on partitions 64:128 -> stage + DMA shift
                yB = npool.tile([64, 512], BF, tag="yB")
                nc.vector.tensor_mul(out=yB, in0=oB_sb[0:64, :], in1=bcB)
                nc.sync.dma_start(out=yT_sb[64:128, fc, ts(Q, 512)], in_=yB)

                if interleave is not None:
                    # slot one c_proj token-block of the previous q-block into
                    # the PE stream here -- the attention phase is exp-bound,
                    # so these matmuls ride in otherwise-idle PE slots
                    proj_tb(interleave, fc)

        def proj_tb(Q, tb):
            trow = Q * 4 + tb
            ps = ps_s.tile([128, 1024], FP32, tag="sAB")
            for ncol in range(NCOL):
                for fc in range(NFC):
                    nc.tensor.matmul(
                        ps[:, ts(ncol, 512)],
                        lhsT=yT_sb[:, fc, ts(trow, 128)],
                        rhs=wp_sb[:, fc, ts(ncol, 512)],
                        start=(fc == 0),
                        stop=(fc == NFC - 1),
                    )
            o_sb = outp.tile([128, 1024], FP32, tag="osb")
            nc.vector.tensor_add(out=o_sb, in0=ps, in1=bp_bc)
            nc.sync.dma_start(out=partial[ds(trow * 128, 128), :], in_=o_sb)

        def rs_block(Q):
            # reduce this 512-token block across the batch pair while later
            # blocks still compute; each core keeps 256 of the 512 rows.
            nc.gpsimd.collective_compute(
                "ReduceScatter",
                mybir.AluOpType.add,
                replica_groups=REPLICA_GROUPS,
                ins=[partial[ds(Q * 512, 512), :]],
                outs=[rs_outs[Q][:]],
            )

        # software pipeline: block Q's c_proj matmuls interleave into the
        # exp-bound attention phase of block Q+1, one token-block per head
        # pair; its ReduceScatter launches right after.
        for Q in range(NQ):
            attention_block(Q, interleave=Q - 1 if Q > 0 else None)
            if Q > 0:
                rs_block(Q - 1)
        for tb in range(4):
            proj_tb(NQ - 1, tb)
        rs_block(NQ - 1)
        # output copies last: by now RS(0..2) are long done; only the final
        # block's wait is real, and nothing is queued behind these
        for Q in range(NQ):
            nc.sync.dma_start(out=out.ap()[ds(Q * 256, 256), :], in_=rs_outs[Q][:])


_NC_CACHE = None


def _get_nc():
    global _NC_CACHE
    if _NC_CACHE is None:
        _NC_CACHE = _build_nc()
    return _NC_CACHE


def kernel(x, w_attn, b_attn, w_proj, b_proj):
    x = np.asarray(x)
    w_attn = np.asarray(w_attn)
    b_attn = np.asarray(b_attn)
    w_proj = np.asarray(w_proj)
    b_proj = np.asarray(b_proj)

    nc = _get_nc()

    in_maps = []
    for i in range(N_CORES):
        b, g = i // 2, i % 2
        cols = slice(g * F, (g + 1) * F)
        in_maps.append(
            {
                "xT": np.ascontiguousarray(x[b].T).astype(BF16),
                "wq": np.ascontiguousarray(w_attn[:, g * F : (g + 1) * F]).astype(BF16),
                "wk": np.ascontiguousarray(
                    w_attn[:, C + g * F : C + (g + 1) * F]
                ).astype(BF16),
                "wv": np.ascontiguousarray(
                    w_attn[:, 2 * C + g * F : 2 * C + (g + 1) * F]
                ).astype(BF16),
                "bq": np.ascontiguousarray(b_attn[g * F : (g + 1) * F]).astype(
                    np.float32
                ),
                "bk": np.ascontiguousarray(b_attn[C + g * F : C + (g + 1) * F]).astype(
                    np.float32
                ),
                "bv": np.ascontiguousarray(
                    b_attn[2 * C + g * F : 2 * C + (g + 1) * F]
                ).astype(np.float32),
                "wp": np.ascontiguousarray(w_proj[g * F : (g + 1) * F, :]).astype(BF16),
                "bp": (b_proj * 0.5).astype(np.float32),
            }
        )

    global _last_in_maps
    _last_in_maps = in_maps  # stashed for external profiling harnesses
    res = run_bass_kernel_spmd(nc, in_maps, core_ids=list(range(N_CORES)))

    # Each core's "out" holds NQ blocks of 256 rows: block Q is the core's
    # ReduceScatter half of token rows [Q*512, (Q+1)*512) -- rank 0 (even
    # core) the first 256, rank 1 (odd core) the last 256.
    out = np.empty((B, T, C), dtype=np.float32)
    for b in range(B):
        even = res.results[2 * b]["out"].reshape(NQ, 256, C)
        odd = res.results[2 * b + 1]["out"].reshape(NQ, 256, C)
        blocks = out[b].reshape(NQ, 2, 256, C)
        blocks[:, 0] = even
        blocks[:, 1] = odd
    return out
